# revision 7
# baseline (speedup 1.0000x reference)
"""Trainium2 Bass kernel for nn_Encoder_38259568672815 (ViT-style encoder).

Strategy: data-parallel over batch (16 images -> 8 cores x 2 images).
On-chip layout: feature-major residual stream [D on partitions, tokens free],
bf16 residual + matmul operands, fp32 PSUM accumulation and LN / softmax
statistics.

Key perf choices vs the v1 kernel:
  - residual kept in bf16 (fp32 moving operands cost 4 cyc/row on PE)
  - softmax denominator folded into the AV matmul via a ones column in V
  - exp processed per (head, kt) over both q-chunks (2-bank PSUM tile)
  - partition broadcasts (softmax 1/d, LN mean/rstd) on the idle Pool engine
  - per-head score matmuls at alternating partition halves (PE row-group
    concurrency for the dk=64 contraction)

Self-contained: hardcodes all shapes; host work is limited to layout
permutations (im2col, weight casts, final transpose) and sharding.
"""
from contextlib import ExitStack

import numpy as np
import ml_dtypes

import concourse.bass as bass
import concourse.tile as tile
import concourse.mybir as mybir
from concourse import bacc
from concourse.masks import make_identity
from concourse.bass_utils import run_bass_kernel_spmd

F32 = mybir.dt.float32
BF16 = mybir.dt.bfloat16
AF = mybir.ActivationFunctionType

B, C, IMG, P = 16, 3, 384, 16
D, NH, DK, L, FF = 768, 12, 64, 6, 3072
S = (IMG // P) ** 2          # 576 tokens per image
NI = 2                       # images per core
T = NI * S                   # 1152 token columns per core
DT = D // 128                # 6 d-tiles
FT = FF // 128               # 24 f-tiles
ST = (S + 127) // 128        # 5 token tiles per image (last = 64)
TCH = 3                      # token chunks of 384 over T
QCH = 2                      # q chunks of 288 per image
QW = S // QCH                # 288
NCORES = 8


def _stiles(img):
    """(kt, row0, ss) k-token tiles for one image."""
    out = []
    for kt in range(ST):
        ss = min(128, S - kt * 128)
        out.append((kt, img * S + kt * 128, ss))
    return out


def build_kernel(n_layers=L):
    nc = bacc.Bacc()

    # ---- DRAM tensors ----
    xp = nc.dram_tensor("xp", [NI, D, S], BF16, kind="ExternalInput")
    wck = nc.dram_tensor("wck", [D, D], BF16, kind="ExternalInput")
    cb = nc.dram_tensor("cb", [D], F32, kind="ExternalInput")
    pef = nc.dram_tensor("pef", [D, S], BF16, kind="ExternalInput")
    wq = nc.dram_tensor("wq", [L, D, D], BF16, kind="ExternalInput")
    wk = nc.dram_tensor("wk", [L, D, D], BF16, kind="ExternalInput")
    wv = nc.dram_tensor("wv", [L, D, D], BF16, kind="ExternalInput")
    wh = nc.dram_tensor("wh", [L, D, D], BF16, kind="ExternalInput")
    whb = nc.dram_tensor("whb", [L, D], F32, kind="ExternalInput")
    ln2s = nc.dram_tensor("ln2s", [L, D], F32, kind="ExternalInput")
    ln2b = nc.dram_tensor("ln2b", [L, D], F32, kind="ExternalInput")
    w1 = nc.dram_tensor("w1", [L, D, FF], BF16, kind="ExternalInput")
    b1 = nc.dram_tensor("b1", [L, FF], F32, kind="ExternalInput")
    w2 = nc.dram_tensor("w2", [L, FF, D], BF16, kind="ExternalInput")
    b2 = nc.dram_tensor("b2", [L, D], F32, kind="ExternalInput")
    lnfs = nc.dram_tensor("lnfs", [D], F32, kind="ExternalInput")
    lnfb = nc.dram_tensor("lnfb", [D], F32, kind="ExternalInput")
    out = nc.dram_tensor("out", [NI, D, S], F32, kind="ExternalOutput")
    scratch = nc.dram_tensor("scratch", [NI, S, D], BF16)

    with tile.TileContext(nc) as tc, ExitStack() as ctx, \
            nc.allow_low_precision(reason="bf16 residual stream by design"):
        xpool = ctx.enter_context(tc.tile_pool(name="x", bufs=1))
        x1pool = ctx.enter_context(tc.tile_pool(name="x1", bufs=1))
        consts = ctx.enter_context(tc.tile_pool(name="consts", bufs=1))
        biasp = ctx.enter_context(tc.tile_pool(name="biasp", bufs=2))

        ones_col = consts.tile([128, 1], BF16)
        nc.vector.memset(ones_col[:], 1.0)
        eps2 = consts.tile([1, 1], F32)
        nc.vector.memset(eps2[:], 1e-6)
        epsf = consts.tile([1, 1], F32)
        nc.vector.memset(epsf[:], 1e-12)
        ident = consts.tile([128, 128], BF16)
        make_identity(nc, ident[:])

        x_sb = xpool.tile([128, DT, T], BF16)
        x1_sb = x1pool.tile([128, DT, T], BF16)

        # ================= Phase A: conv patch embedding =================
        with tc.tile_pool(name="conv", bufs=1) as convp, \
             tc.tile_pool(name="cps", bufs=3, space="PSUM") as cps, \
             tc.tile_pool(name="emb", bufs=2) as embp:
            wck_sb = convp.tile([128, DT, D], BF16)
            nc.sync.dma_start(wck_sb[:], wck.rearrange("(t p) d -> p t d", p=128))
            cb_sb = convp.tile([128, DT], F32)
            nc.sync.dma_start(cb_sb[:], cb.rearrange("(t p) -> p t", p=128))
            xp_sb = convp.tile([128, NI, DT, S], BF16)
            nc.sync.dma_start(xp_sb[:], xp.rearrange("b (t p) s -> p b t s", p=128))
            for img in range(NI):
                emb_sb = embp.tile([128, DT, S], BF16)
                for dm in range(DT):
                    for ch in range(QCH):
                        ps = cps.tile([128, QW], F32)
                        for kt in range(DT):
                            nc.tensor.matmul(
                                ps[:],
                                wck_sb[:, kt, dm * 128:(dm + 1) * 128],
                                xp_sb[:, img, kt, ch * QW:(ch + 1) * QW],
                                start=(kt == 0), stop=(kt == DT - 1))
                        nc.scalar.activation(
                            emb_sb[:, dm, ch * QW:(ch + 1) * QW], ps[:],
                            AF.Tanh, bias=cb_sb[:, dm:dm + 1])
                # write d-major flat: flat[(d, s)] with d = t*128 + p
                nc.sync.dma_start(
                    scratch[img].rearrange("s d -> (s d)").rearrange(
                        "(t p s) -> p t s", p=128, s=S),
                    emb_sb[:])

        # ============ Phase B: reshape quirk + pos-enc -> x (bf16) ============
        with tc.tile_pool(name="htok", bufs=3) as hp, \
             tc.tile_pool(name="tps", bufs=4, space="PSUM") as tps, \
             tc.tile_pool(name="pe", bufs=1) as pep:
            pe_sb = pep.tile([128, DT, S], BF16)
            nc.sync.dma_start(pe_sb[:], pef.rearrange("(t p) s -> p t s", p=128))
            for img in range(NI):
                for st in range(ST):
                    ss = min(128, S - st * 128)
                    h_sb = hp.tile([128, D], BF16)
                    nc.sync.dma_start(h_sb[:ss, :],
                                      scratch[img, st * 128:st * 128 + ss, :])
                    for dtile in range(DT):
                        pst = tps.tile([128, 128], BF16)
                        nc.tensor.transpose(
                            pst[:, 0:ss], h_sb[:ss, dtile * 128:(dtile + 1) * 128],
                            ident[0:ss, 0:ss])
                        nc.vector.tensor_add(
                            x_sb[:, dtile, img * S + st * 128: img * S + st * 128 + ss],
                            pst[:, 0:ss], pe_sb[:, dtile, st * 128:st * 128 + ss])

        # phase boundary: keep conv/permute pools from overlapping layer pools
        tc.strict_bb_all_engine_barrier()

        # layer-phase pools (opened after conv pools close to fit SBUF)
        wqkv = ctx.enter_context(tc.tile_pool(name="wqkv", bufs=4))
        qkp = ctx.enter_context(tc.tile_pool(name="qk", bufs=2))
        vp = ctx.enter_context(tc.tile_pool(name="v", bufs=2))
        ep = ctx.enter_context(tc.tile_pool(name="E", bufs=2))
        hvp = ctx.enter_context(tc.tile_pool(name="hv", bufs=2))
        smallp = ctx.enter_context(tc.tile_pool(name="small", bufs=1))
        rp = ctx.enter_context(tc.tile_pool(name="r", bufs=2))
        bcp = ctx.enter_context(tc.tile_pool(name="bc", bufs=2))
        xnp = ctx.enter_context(tc.tile_pool(name="xn", bufs=1))
        ffw = ctx.enter_context(tc.tile_pool(name="ffw", bufs=4))
        gp = ctx.enter_context(tc.tile_pool(name="g", bufs=2))
        tmpp = ctx.enter_context(tc.tile_pool(name="tmp", bufs=2))

        # ================= Phase C: encoder layers =================
        cur, nxt = x_sb, x1_sb
        for li in range(n_layers):
            wq_sb = wqkv.tile([128, DT, D], BF16, tag="w4")
            nc.sync.dma_start(wq_sb[:], wq[li].rearrange("(t p) e -> p t e", p=128))
            wk_sb = wqkv.tile([128, DT, D], BF16, tag="w4")
            nc.sync.dma_start(wk_sb[:], wk[li].rearrange("(t p) e -> p t e", p=128))
            wv_sb = wqkv.tile([128, DT, D], BF16, tag="w4")
            nc.sync.dma_start(wv_sb[:], wv[li].rearrange("(t p) e -> p t e", p=128))
            whb_sb = biasp.tile([128, DT], F32, tag="whb")
            nc.sync.dma_start(whb_sb[:], whb[li].rearrange("(t p) -> p t", p=128))
            l2s_sb = biasp.tile([128, DT], F32, tag="l2s")
            nc.sync.dma_start(l2s_sb[:], ln2s[li].rearrange("(t p) -> p t", p=128))
            l2b_sb = biasp.tile([128, DT], F32, tag="l2b")
            nc.sync.dma_start(l2b_sb[:], ln2b[li].rearrange("(t p) -> p t", p=128))
            b1_sb = biasp.tile([128, FT], F32, tag="b1")
            nc.sync.dma_start(b1_sb[:], b1[li].rearrange("(t p) -> p t", p=128))
            b2_sb = biasp.tile([128, DT], F32, tag="b2")
            nc.sync.dma_start(b2_sb[:], b2[li].rearrange("(t p) -> p t", p=128))
            wh_sb = wqkv.tile([128, DT, D], BF16, tag="w4")
            nc.sync.dma_start(wh_sb[:], wh[li].rearrange("(t p) e -> p t e", p=128))

            with tc.tile_pool(name="pps", bufs=2, space="PSUM") as pps, \
                 tc.tile_pool(name="scps", bufs=2, space="PSUM") as scps, \
                 tc.tile_pool(name="hps", bufs=2, space="PSUM") as hps:
              for img in range(NI):
                # ---- C1: Q,K projections (feature-major, this image) ----
                qk_i = qkp.tile([128, 2 * DT, S], BF16, tag="qk")
                v65 = vp.tile([128, ST, NH // 2, 192], BF16, tag="v")
                for mi, w_sb in ((0, wq_sb), (1, wk_sb)):
                    for mt in range(DT):
                        for ch in range(QCH):
                            ps = pps.tile([128, 384], F32, name="qkps", tag="p")
                            for kt in range(DT):
                                nc.tensor.matmul(
                                    ps[:, 0:QW],
                                    w_sb[:, kt, mt * 128:(mt + 1) * 128],
                                    cur[:, kt, img * S + ch * QW: img * S + (ch + 1) * QW],
                                    start=(kt == 0), stop=(kt == DT - 1))
                            nc.vector.tensor_copy(
                                qk_i[:, mi * DT + mt, ch * QW:(ch + 1) * QW],
                                ps[:, 0:QW])
                # ---- C2: V projection (token-major) ----
                # per head pair: [V_even(64) | 1 | junk(63) | V_odd(64)].
                # Even-head AV uses stationary [V_e|1] (65 cols): d at out
                # row 64. Odd-head AV uses [1|junk|V_o] (128 cols): d at out
                # row 0, hv at rows 64:128 (junk rows 1:64 never read).
                for (kt, row0, ss) in _stiles(img):
                    nc.gpsimd.memset(v65[:, kt, :, 64:65], 1.0)
                    for ch2 in range(2):
                        ps = pps.tile([128, 384], F32, name="vps", tag="p")
                        for dti in range(DT):
                            nc.tensor.matmul(
                                ps[:ss, :],
                                cur[:, dti, row0:row0 + ss],
                                wv_sb[:, dti, ch2 * 384:(ch2 + 1) * 384],
                                start=(dti == 0), stop=(dti == DT - 1))
                        pr0 = 3 * ch2
                        nc.vector.tensor_copy(
                            v65[:ss, kt, pr0:pr0 + 3, 0:64],
                            ps[:ss].rearrange("p (h c) -> p h c", h=6)[:, 0:6:2, :])
                        nc.vector.tensor_copy(
                            v65[:ss, kt, pr0:pr0 + 3, 128:192],
                            ps[:ss].rearrange("p (h c) -> p h c", h=6)[:, 1:6:2, :])

                # ---- C3: attention (this image) ----
                hv_i = hvp.tile([128, DT, S], BF16, tag="hv")
                for hpair in range(NH // 2):
                    e_pair = []
                    for h01 in range(2):
                        h = 2 * hpair + h01
                        p0 = h01 * 64
                        e_t = ep.tile([128, ST, QCH, QW], BF16, tag="E",
                                      name=f"E_{h01}")
                        e_pair.append(e_t)
                        for (kt, row0, ss) in _stiles(0):
                            sc = scps.tile([128, 2, 512], F32, name="scps",
                                           tag="s")
                            for qc in range(QCH):
                                nc.tensor.matmul(
                                    sc[0:ss, qc, 0:QW],
                                    qk_i[p0:p0 + 64, DT + hpair,
                                         kt * 128:kt * 128 + ss],
                                    qk_i[p0:p0 + 64, hpair,
                                         qc * QW:(qc + 1) * QW],
                                    start=True, stop=True)
                            nc.scalar.activation(
                                e_t[0:ss, kt, :, :], sc[0:ss, :, 0:QW],
                                AF.Exp, scale=0.125)
                    for h01 in range(2):
                        e_t = e_pair[h01]
                        p0 = h01 * 64
                        # even head: stationary [V|1] -> rows 0:64 hv, 64 d
                        # odd head: stationary [1|junk|V] -> row 0 d,
                        #           rows 64:128 hv
                        nr = 65 if h01 == 0 else 128
                        c0 = 0 if h01 == 0 else 64
                        dr = 64 if h01 == 0 else 0
                        for qc in range(QCH):
                            hv_ps = hps.tile([128, 512], F32, name="hvps",
                                             tag="h")
                            for (kt, row0, ss) in _stiles(0):
                                nc.tensor.matmul(
                                    hv_ps[0:nr, 0:QW],
                                    v65[0:ss, kt, hpair, c0:c0 + nr],
                                    e_t[0:ss, kt, qc, :],
                                    start=(kt == 0), stop=(kt == ST - 1))
                            r_sb = rp.tile([128, QW], F32, tag="rsb")
                            nc.vector.reciprocal(r_sb[dr:dr + 1, :],
                                                 hv_ps[dr:dr + 1, 0:QW])
                            # partition_broadcast HW ucode reads/writes from
                            # absolute partition 0 (AP partition offsets are
                            # ignored) - shift the even head's d row down first
                            if dr != 0:
                                r0 = rp.tile([1, QW], F32, tag="r0")
                                nc.sync.dma_start(r0[0:1, :],
                                                  r_sb[dr:dr + 1, :])
                                src = r0[0:1, :]
                            else:
                                src = r_sb[0:1, :]
                            r_b = bcp.tile([128, QW], F32, tag="rb")
                            nc.gpsimd.partition_broadcast(
                                r_b[0:p0 + 64, :], src, channels=p0 + 64)
                            nc.vector.tensor_mul(
                                hv_i[p0:p0 + 64, hpair, qc * QW:(qc + 1) * QW],
                                hv_ps[p0:p0 + 64, 0:QW],
                                r_b[p0:p0 + 64, :])

                # ---- C4: Wh + bias + residual -> nxt (this image) ----
                for mt in range(DT):
                    for ch in range(QCH):
                        ps = pps.tile([128, 384], F32, name="ops", tag="p")
                        for et in range(DT):
                            nc.tensor.matmul(
                                ps[:, 0:QW], wh_sb[:, et, mt * 128:(mt + 1) * 128],
                                hv_i[:, et, ch * QW:(ch + 1) * QW],
                                start=(et == 0), stop=(et == DT - 1))
                        t_f = tmpp.tile([128, QW], BF16, tag="t46")
                        nc.scalar.activation(t_f[:], ps[:, 0:QW], AF.Identity,
                                             bias=whb_sb[:, mt:mt + 1])
                        nc.vector.tensor_add(
                            nxt[:, mt, img * S + ch * QW: img * S + (ch + 1) * QW],
                            t_f[:],
                            cur[:, mt, img * S + ch * QW: img * S + (ch + 1) * QW])

            # ---- C5: LayerNorm(nxt) -> xn (bf16) ----
            xn = xnp.tile([128, DT, T], BF16, tag="xn")
            with tc.tile_pool(name="stps", bufs=1, space="PSUM") as stps:
                for ch in range(TCH):
                    sq = xnp.tile([128, DT, 384], BF16, tag="sq")
                    for kt in range(DT):
                        nc.vector.tensor_mul(sq[:, kt, :],
                                             nxt[:, kt, ch * 384:(ch + 1) * 384],
                                             nxt[:, kt, ch * 384:(ch + 1) * 384])
                    st0 = stps.tile([1, 384], F32, tag="st0")
                    st1 = stps.tile([1, 384], F32, tag="st1")
                    for kt in range(DT):
                        nc.tensor.matmul(st0[:], ones_col[:],
                                         nxt[:, kt, ch * 384:(ch + 1) * 384],
                                         start=(kt == 0), stop=(kt == DT - 1))
                        nc.tensor.matmul(st1[:], ones_col[:],
                                         sq[:, kt, :],
                                         start=(kt == 0), stop=(kt == DT - 1))
                    mom = smallp.tile([1, 384], F32, tag="mom")
                    nc.scalar.mul(mom[:], st0[:], 1.0 / D)
                    mom_bf = smallp.tile([1, 384], BF16, tag="mombf")
                    nc.vector.tensor_copy(mom_bf[:], mom[:])
                    msq = smallp.tile([1, 384], F32, tag="msq")
                    nc.vector.tensor_mul(msq[:], mom[:], mom[:])
                    ex2 = smallp.tile([1, 384], F32, tag="ex2")
                    nc.scalar.mul(ex2[:], st1[:], 1.0 / D)
                    var = smallp.tile([1, 384], F32, tag="var")
                    nc.vector.tensor_sub(var[:], ex2[:], msq[:])
                    nc.scalar.activation(var[:], var[:], AF.Sqrt, bias=eps2[:])
                    rstd = smallp.tile([1, 384], F32, tag="rstd")
                    nc.vector.reciprocal(rstd[:], var[:])
                    rstd_bf = smallp.tile([1, 384], BF16, tag="rstdbf")
                    nc.vector.tensor_copy(rstd_bf[:], rstd[:])
                    m_b = bcp.tile([128, 384], BF16, tag="mb")
                    nc.gpsimd.partition_broadcast(m_b[:], mom_bf[0:1, :])
                    r_b = bcp.tile([128, 384], BF16, tag="rb2")
                    nc.gpsimd.partition_broadcast(r_b[:], rstd_bf[0:1, :])
                    for mt in range(DT):
                        t_c = tmpp.tile([128, 384], BF16, tag="t5a")
                        nc.vector.tensor_sub(t_c[:],
                                             nxt[:, mt, ch * 384:(ch + 1) * 384],
                                             m_b[:])
                        t_d = tmpp.tile([128, 384], BF16, tag="t5b")
                        nc.vector.tensor_mul(t_d[:], t_c[:], r_b[:])
                        nc.scalar.activation(
                            xn[:, mt, ch * 384:(ch + 1) * 384], t_d[:],
                            AF.Identity, bias=l2b_sb[:, mt:mt + 1],
                            scale=l2s_sb[:, mt:mt + 1])

            # ---- C6: FFN + residual (in place on nxt) ----
            with tc.tile_pool(name="f2ps", bufs=1, space="PSUM") as f2ps, \
                 tc.tile_pool(name="gps", bufs=2, space="PSUM") as gps:
                for tch in range(TCH):
                    f2 = [f2ps.tile([128, 384], F32, tag=f"f2_{mt}", name=f"f2_{mt}")
                          for mt in range(DT)]
                    for ft in range(FT):
                        w1_sb = ffw.tile([128, DT, 128], BF16, tag="w1")
                        nc.sync.dma_start(
                            w1_sb[:],
                            w1[li, :, ft * 128:(ft + 1) * 128].rearrange(
                                "(t p) f -> p t f", p=128))
                        w2_sb = ffw.tile([128, D], BF16, tag="w2")
                        nc.sync.dma_start(w2_sb[:], w2[li, ft * 128:(ft + 1) * 128, :])
                        g_ps = gps.tile([128, 384], F32)
                        for kt in range(DT):
                            nc.tensor.matmul(
                                g_ps[:], w1_sb[:, kt, :],
                                xn[:, kt, tch * 384:(tch + 1) * 384],
                                start=(kt == 0), stop=(kt == DT - 1))
                        g_bf = gp.tile([128, 384], BF16, tag="gbf")
                        nc.scalar.activation(g_bf[:], g_ps[:], AF.Gelu,
                                             bias=b1_sb[:, ft:ft + 1])
                        for mt in range(DT):
                            nc.tensor.matmul(
                                f2[mt][:], w2_sb[:, mt * 128:(mt + 1) * 128],
                                g_bf[:], start=(ft == 0), stop=(ft == FT - 1))
                    for mt in range(DT):
                        t_f = tmpp.tile([128, 384], BF16, tag="t46b")
                        nc.scalar.activation(t_f[:], f2[mt][:], AF.Identity,
                                             bias=b2_sb[:, mt:mt + 1])
                        nc.vector.tensor_add(
                            nxt[:, mt, tch * 384:(tch + 1) * 384],
                            t_f[:], nxt[:, mt, tch * 384:(tch + 1) * 384])
            cur, nxt = nxt, cur

        # ================= Final LayerNorm -> out =================
        lnf_s = biasp.tile([128, DT], F32, tag="lnfs")
        nc.sync.dma_start(lnf_s[:], lnfs.rearrange("(t p) -> p t", p=128))
        lnf_b = biasp.tile([128, DT], F32, tag="lnfb")
        nc.sync.dma_start(lnf_b[:], lnfb.rearrange("(t p) -> p t", p=128))
        with tc.tile_pool(name="fout", bufs=2) as foutp, \
             tc.tile_pool(name="fstps", bufs=1, space="PSUM") as stps:
            for ch in range(TCH):
                sqf = xnp.tile([128, DT, 384], BF16, tag="sq")
                for kt in range(DT):
                    nc.vector.tensor_mul(sqf[:, kt, :],
                                         cur[:, kt, ch * 384:(ch + 1) * 384],
                                         cur[:, kt, ch * 384:(ch + 1) * 384])
                st0 = stps.tile([1, 384], F32, tag="st0")
                st1 = stps.tile([1, 384], F32, tag="st1")
                for kt in range(DT):
                    nc.tensor.matmul(st0[:], ones_col[:],
                                     cur[:, kt, ch * 384:(ch + 1) * 384],
                                     start=(kt == 0), stop=(kt == DT - 1))
                    nc.tensor.matmul(st1[:], ones_col[:],
                                     sqf[:, kt, :],
                                     start=(kt == 0), stop=(kt == DT - 1))
                mom = smallp.tile([1, 384], F32, tag="mom")
                nc.scalar.mul(mom[:], st0[:], 1.0 / D)
                mom_bf = smallp.tile([1, 384], BF16, tag="mombf")
                nc.vector.tensor_copy(mom_bf[:], mom[:])
                msq = smallp.tile([1, 384], F32, tag="msq")
                nc.vector.tensor_mul(msq[:], mom[:], mom[:])
                ex2 = smallp.tile([1, 384], F32, tag="ex2")
                nc.scalar.mul(ex2[:], st1[:], 1.0 / D)
                var = smallp.tile([1, 384], F32, tag="var")
                nc.vector.tensor_sub(var[:], ex2[:], msq[:])
                nc.scalar.activation(var[:], var[:], AF.Sqrt, bias=epsf[:])
                rstd = smallp.tile([1, 384], F32, tag="rstd")
                nc.vector.reciprocal(rstd[:], var[:])
                rstd_bf = smallp.tile([1, 384], BF16, tag="rstdbf")
                nc.vector.tensor_copy(rstd_bf[:], rstd[:])
                m_b = bcp.tile([128, 384], BF16, tag="mb")
                nc.gpsimd.partition_broadcast(m_b[:], mom_bf[0:1, :])
                r_b = bcp.tile([128, 384], BF16, tag="rb2")
                nc.gpsimd.partition_broadcast(r_b[:], rstd_bf[0:1, :])
                for mt in range(DT):
                    t_c = tmpp.tile([128, 384], BF16, tag="t5a")
                    nc.vector.tensor_sub(t_c[:],
                                         cur[:, mt, ch * 384:(ch + 1) * 384], m_b[:])
                    t_d = tmpp.tile([128, 384], BF16, tag="t5b")
                    nc.vector.tensor_mul(t_d[:], t_c[:], r_b[:])
                    o_sb = foutp.tile([128, 384], F32)
                    nc.scalar.activation(o_sb[:], t_d[:], AF.Identity,
                                         bias=lnf_b[:, mt:mt + 1],
                                         scale=lnf_s[:, mt:mt + 1])
                    c0 = ch * 384
                    for off in range(0, 384, 192):
                        col = c0 + off
                        img, s0 = divmod(col, S)
                        nc.sync.dma_start(
                            out[img, mt * 128:(mt + 1) * 128, s0:s0 + 192],
                            o_sb[:, off:off + 192])
    nc.finalize()
    return nc


def _pos_encoding(max_len, d):
    pos = np.arange(max_len)[:, None].astype(np.float32)
    div = np.exp(np.arange(0, d, 2).astype(np.float32) * (-np.log(10000.0) / d))
    pe = np.zeros((max_len, d), dtype=np.float32)
    pe[:, 0::2] = np.sin(pos * div)
    pe[:, 1::2] = np.cos(pos * div)
    return pe


_NC_CACHE = {}


def get_nc(n_layers=L):
    if n_layers not in _NC_CACHE:
        _NC_CACHE[n_layers] = build_kernel(n_layers)
    return _NC_CACHE[n_layers]


def make_in_maps(x, conv_w, conv_b, ln1_s, ln1_b, wq, wk, wv, wh, wh_b,
                 ln2_s, ln2_b, w1, b1, w2, b2, lnf_s, lnf_b):
    bf = ml_dtypes.bfloat16
    x = np.asarray(x, np.float32)
    patches = x.reshape(B, C, IMG // P, P, IMG // P, P)      # (B,C,ty,py,tx,px)
    patches = patches.transpose(0, 1, 3, 5, 2, 4).reshape(B, D, S).astype(bf)
    wckh = np.ascontiguousarray(
        np.asarray(conv_w, np.float32).reshape(D, D).T).astype(bf)
    pefh = np.ascontiguousarray(_pos_encoding(5000, D)[:S].T).astype(bf)
    shared = {
        "wck": wckh, "cb": np.asarray(conv_b, np.float32), "pef": pefh,
        "wq": np.asarray(wq, np.float32).astype(bf),
        "wk": np.asarray(wk, np.float32).astype(bf),
        "wv": np.asarray(wv, np.float32).astype(bf),
        "wh": np.asarray(wh, np.float32).astype(bf),
        "whb": np.asarray(wh_b, np.float32),
        "ln2s": np.asarray(ln2_s, np.float32),
        "ln2b": np.asarray(ln2_b, np.float32),
        "w1": np.asarray(w1, np.float32).astype(bf),
        "b1": np.asarray(b1, np.float32),
        "w2": np.asarray(w2, np.float32).astype(bf),
        "b2": np.asarray(b2, np.float32),
        "lnfs": np.asarray(lnf_s, np.float32),
        "lnfb": np.asarray(lnf_b, np.float32),
    }
    in_maps = []
    for c in range(NCORES):
        m = dict(shared)
        m["xp"] = np.ascontiguousarray(patches[c * NI:(c + 1) * NI])
        in_maps.append(m)
    return in_maps


def assemble_output(results):
    out = np.empty((B, S, D), np.float32)
    for c in range(NCORES):
        o = results[c]["out"]
        for i in range(NI):
            out[c * NI + i] = o[i].T
    return out


def kernel(**inputs) -> np.ndarray:
    nc = get_nc()
    in_maps = make_in_maps(**inputs)
    res = run_bass_kernel_spmd(nc, in_maps, core_ids=list(range(NCORES)))
    return assemble_output(res.results)


# revision 13
# speedup vs baseline: 1.2531x; 1.2531x over previous
"""Trainium2 Bass kernel for nn_Encoder_38259568672815 (ViT-style encoder).

Strategy: data-parallel over batch (16 images -> 8 cores x 2 images).
On-chip layout: feature-major residual stream [D on partitions, tokens free],
bf16 residual + matmul operands, fp32 PSUM accumulation and LN / softmax
statistics.

Key perf choices vs the v1 kernel:
  - residual kept in bf16 (fp32 moving operands cost 4 cyc/row on PE)
  - softmax denominator folded into the AV matmul via a ones column in V
  - exp processed per (head, kt) over both q-chunks (2-bank PSUM tile)
  - partition broadcasts (softmax 1/d, LN mean/rstd) on the idle Pool engine
  - per-head score matmuls at alternating partition halves (PE row-group
    concurrency for the dk=64 contraction)

Self-contained: hardcodes all shapes; host work is limited to layout
permutations (im2col, weight casts, final transpose) and sharding.
"""
from contextlib import ExitStack

import numpy as np
import ml_dtypes

import concourse.bass as bass
import concourse.tile as tile
import concourse.mybir as mybir
from concourse import bacc
from concourse.masks import make_identity
from concourse.bass_utils import run_bass_kernel_spmd

F32 = mybir.dt.float32
F32R = mybir.dt.float32r
BF16 = mybir.dt.bfloat16
AF = mybir.ActivationFunctionType

B, C, IMG, P = 16, 3, 384, 16
D, NH, DK, L, FF = 768, 12, 64, 6, 3072
S = (IMG // P) ** 2          # 576 tokens per image
NI = 2                       # images per core
T = NI * S                   # 1152 token columns per core
DT = D // 128                # 6 d-tiles
FT = FF // 128               # 24 f-tiles
ST = (S + 127) // 128        # 5 token tiles per image (last = 64)
TCH = 3                      # token chunks of 384 over T
QCH = 2                      # q chunks of 288 per image
QW = S // QCH                # 288
NCORES = 8


def _stiles(img):
    """(kt, row0, ss) k-token tiles for one image."""
    out = []
    for kt in range(ST):
        ss = min(128, S - kt * 128)
        out.append((kt, img * S + kt * 128, ss))
    return out


def build_kernel(n_layers=L):
    nc = bacc.Bacc()

    # ---- DRAM tensors ----
    xp = nc.dram_tensor("xp", [NI, D, S], BF16, kind="ExternalInput")
    wck = nc.dram_tensor("wck", [D, D], BF16, kind="ExternalInput")
    cb = nc.dram_tensor("cb", [D], F32, kind="ExternalInput")
    pef = nc.dram_tensor("pef", [D, S], F32, kind="ExternalInput")
    wq = nc.dram_tensor("wq", [L, D, D], BF16, kind="ExternalInput")
    wk = nc.dram_tensor("wk", [L, D, D], BF16, kind="ExternalInput")
    wv = nc.dram_tensor("wv", [L, D, D], BF16, kind="ExternalInput")
    wh = nc.dram_tensor("wh", [L, D, D], BF16, kind="ExternalInput")
    whb = nc.dram_tensor("whb", [L, D], F32, kind="ExternalInput")
    ln2s = nc.dram_tensor("ln2s", [L, D], F32, kind="ExternalInput")
    ln2b = nc.dram_tensor("ln2b", [L, D], F32, kind="ExternalInput")
    w1 = nc.dram_tensor("w1", [L, D, FF], BF16, kind="ExternalInput")
    b1 = nc.dram_tensor("b1", [L, FF], F32, kind="ExternalInput")
    w2 = nc.dram_tensor("w2", [L, FF, D], BF16, kind="ExternalInput")
    b2 = nc.dram_tensor("b2", [L, D], F32, kind="ExternalInput")
    lnfs = nc.dram_tensor("lnfs", [D], F32, kind="ExternalInput")
    lnfb = nc.dram_tensor("lnfb", [D], F32, kind="ExternalInput")
    out = nc.dram_tensor("out", [NI, D, S], F32, kind="ExternalOutput")
    scratch = nc.dram_tensor("scratch", [NI, S, D], BF16)

    with tile.TileContext(nc) as tc, ExitStack() as ctx, \
            nc.allow_low_precision(reason="bf16 residual stream by design"):
        xpool = ctx.enter_context(tc.tile_pool(name="x", bufs=1))
        xbp = ctx.enter_context(tc.tile_pool(name="xb", bufs=1))
        consts = ctx.enter_context(tc.tile_pool(name="consts", bufs=1))
        biasp = ctx.enter_context(tc.tile_pool(name="biasp", bufs=2))

        ones_col = consts.tile([128, 1], BF16)
        nc.vector.memset(ones_col[:], 1.0)
        eps2 = consts.tile([1, 1], F32)
        nc.vector.memset(eps2[:], 1e-6)
        epsf = consts.tile([1, 1], F32)
        nc.vector.memset(epsf[:], 1e-12)
        ident = consts.tile([128, 128], BF16)
        make_identity(nc, ident[:])

        res = xpool.tile([128, DT, T], F32R)
        ones_f32 = consts.tile([128, 1], F32)
        nc.vector.memset(ones_f32[:], 1.0)
        ones_f = consts.tile([128, 1], F32R)
        nc.vector.tensor_copy(ones_f[:], ones_f32[:])

        # ================= Phase A: conv patch embedding =================
        with tc.tile_pool(name="conv", bufs=1) as convp, \
             tc.tile_pool(name="cps", bufs=3, space="PSUM") as cps, \
             tc.tile_pool(name="emb", bufs=2) as embp:
            wck_sb = convp.tile([128, DT, D], BF16)
            nc.sync.dma_start(wck_sb[:], wck.rearrange("(t p) d -> p t d", p=128))
            cb_sb = convp.tile([128, DT], F32)
            nc.sync.dma_start(cb_sb[:], cb.rearrange("(t p) -> p t", p=128))
            xp_sb = convp.tile([128, NI, DT, S], BF16)
            nc.sync.dma_start(xp_sb[:], xp.rearrange("b (t p) s -> p b t s", p=128))
            for img in range(NI):
                emb_sb = embp.tile([128, DT, S], BF16)
                for dm in range(DT):
                    for ch in range(QCH):
                        ps = cps.tile([128, QW], F32)
                        for kt in range(DT):
                            nc.tensor.matmul(
                                ps[:],
                                wck_sb[:, kt, dm * 128:(dm + 1) * 128],
                                xp_sb[:, img, kt, ch * QW:(ch + 1) * QW],
                                start=(kt == 0), stop=(kt == DT - 1))
                        nc.scalar.activation(
                            emb_sb[:, dm, ch * QW:(ch + 1) * QW], ps[:],
                            AF.Tanh, bias=cb_sb[:, dm:dm + 1])
                # write d-major flat: flat[(d, s)] with d = t*128 + p
                nc.sync.dma_start(
                    scratch[img].rearrange("s d -> (s d)").rearrange(
                        "(t p s) -> p t s", p=128, s=S),
                    emb_sb[:])

        # ============ Phase B: reshape quirk + pos-enc -> x (bf16) ============
        with tc.tile_pool(name="htok", bufs=3) as hp, \
             tc.tile_pool(name="tps", bufs=4, space="PSUM") as tps, \
             tc.tile_pool(name="pe", bufs=1) as pep:
            pe_sb = pep.tile([128, DT, S], F32)
            nc.sync.dma_start(pe_sb[:], pef.rearrange("(t p) s -> p t s", p=128))
            for img in range(NI):
                for st in range(ST):
                    ss = min(128, S - st * 128)
                    h_sb = hp.tile([128, D], BF16)
                    nc.sync.dma_start(h_sb[:ss, :],
                                      scratch[img, st * 128:st * 128 + ss, :])
                    for dtile in range(DT):
                        pst = tps.tile([128, 128], BF16)
                        nc.tensor.transpose(
                            pst[:, 0:ss], h_sb[:ss, dtile * 128:(dtile + 1) * 128],
                            ident[0:ss, 0:ss])
                        nc.vector.tensor_add(
                            res[:, dtile, img * S + st * 128: img * S + st * 128 + ss],
                            pst[:, 0:ss], pe_sb[:, dtile, st * 128:st * 128 + ss])

        # phase boundary: keep conv/permute pools from overlapping layer pools
        tc.strict_bb_all_engine_barrier()

        # layer-phase pools (opened after conv pools close to fit SBUF)
        wqkv = ctx.enter_context(tc.tile_pool(name="wqkv", bufs=4))
        qkp = ctx.enter_context(tc.tile_pool(name="qk", bufs=2))
        vp = ctx.enter_context(tc.tile_pool(name="v", bufs=1))
        ep = ctx.enter_context(tc.tile_pool(name="E", bufs=2))
        hvp = ctx.enter_context(tc.tile_pool(name="hv", bufs=2))
        smallp = ctx.enter_context(tc.tile_pool(name="small", bufs=1))
        rp = ctx.enter_context(tc.tile_pool(name="r", bufs=2))
        bcp = ctx.enter_context(tc.tile_pool(name="bc", bufs=2))
        xnp = ctx.enter_context(tc.tile_pool(name="xn", bufs=1))
        ffw = ctx.enter_context(tc.tile_pool(name="ffw", bufs=4))
        gp = ctx.enter_context(tc.tile_pool(name="g", bufs=2))
        tmpp = ctx.enter_context(tc.tile_pool(name="tmp", bufs=2))

        # ================= Phase C: encoder layers =================
        for li in range(n_layers):
            xb = xbp.tile([128, DT, T], BF16, tag="xb")
            for dtile in range(DT):
                nc.vector.tensor_copy(xb[:, dtile, :], res[:, dtile, :])
            wq_sb = wqkv.tile([128, DT, D], BF16, tag="w4")
            nc.sync.dma_start(wq_sb[:], wq[li].rearrange("(t p) e -> p t e", p=128))
            wk_sb = wqkv.tile([128, DT, D], BF16, tag="w4")
            nc.sync.dma_start(wk_sb[:], wk[li].rearrange("(t p) e -> p t e", p=128))
            wv_sb = wqkv.tile([128, DT, D], BF16, tag="w4")
            nc.sync.dma_start(wv_sb[:], wv[li].rearrange("(t p) e -> p t e", p=128))
            whb_sb = biasp.tile([128, DT], F32, tag="whb")
            nc.sync.dma_start(whb_sb[:], whb[li].rearrange("(t p) -> p t", p=128))
            l2s_sb = biasp.tile([128, DT], F32, tag="l2s")
            nc.sync.dma_start(l2s_sb[:], ln2s[li].rearrange("(t p) -> p t", p=128))
            l2b_sb = biasp.tile([128, DT], F32, tag="l2b")
            nc.sync.dma_start(l2b_sb[:], ln2b[li].rearrange("(t p) -> p t", p=128))
            b1_sb = biasp.tile([128, FT], F32, tag="b1")
            nc.sync.dma_start(b1_sb[:], b1[li].rearrange("(t p) -> p t", p=128))
            b2_sb = biasp.tile([128, DT], F32, tag="b2")
            nc.sync.dma_start(b2_sb[:], b2[li].rearrange("(t p) -> p t", p=128))
            wh_sb = wqkv.tile([128, DT, D], BF16, tag="w4")
            nc.sync.dma_start(wh_sb[:], wh[li].rearrange("(t p) e -> p t e", p=128))

            with tc.tile_pool(name="pps", bufs=2, space="PSUM") as pps, \
                 tc.tile_pool(name="scps", bufs=2, space="PSUM") as scps, \
                 tc.tile_pool(name="hps", bufs=2, space="PSUM") as hps:
              for img in range(NI):
                # ---- C1: Q,K projections (feature-major, this image) ----
                qk_i = qkp.tile([128, 2 * DT, S], BF16, tag="qk")
                v65 = vp.tile([128, ST, NH // 2, 192], BF16, tag="v")
                for mi, w_sb in ((0, wq_sb), (1, wk_sb)):
                    for mt in range(DT):
                        for ch in range(QCH):
                            ps = pps.tile([128, 384], F32, name="qkps", tag="p")
                            for kt in range(DT):
                                nc.tensor.matmul(
                                    ps[:, 0:QW],
                                    w_sb[:, kt, mt * 128:(mt + 1) * 128],
                                    xb[:, kt, img * S + ch * QW: img * S + (ch + 1) * QW],
                                    start=(kt == 0), stop=(kt == DT - 1))
                            nc.vector.tensor_copy(
                                qk_i[:, mi * DT + mt, ch * QW:(ch + 1) * QW],
                                ps[:, 0:QW])
                # ---- C2: V projection (token-major) ----
                # per head pair: [V_even(64) | 1 | junk(63) | V_odd(64)].
                # Even-head AV uses stationary [V_e|1] (65 cols): d at out
                # row 64. Odd-head AV uses [1|junk|V_o] (128 cols): d at out
                # row 0, hv at rows 64:128 (junk rows 1:64 never read).
                for (kt, row0, ss) in _stiles(img):
                    nc.gpsimd.memset(v65[:, kt, :, 64:65], 1.0)
                    for ch2 in range(2):
                        ps = pps.tile([128, 384], F32, name="vps", tag="p")
                        for dti in range(DT):
                            nc.tensor.matmul(
                                ps[:ss, :],
                                xb[:, dti, row0:row0 + ss],
                                wv_sb[:, dti, ch2 * 384:(ch2 + 1) * 384],
                                start=(dti == 0), stop=(dti == DT - 1))
                        pr0 = 3 * ch2
                        nc.vector.tensor_copy(
                            v65[:ss, kt, pr0:pr0 + 3, 0:64],
                            ps[:ss].rearrange("p (h c) -> p h c", h=6)[:, 0:6:2, :])
                        nc.vector.tensor_copy(
                            v65[:ss, kt, pr0:pr0 + 3, 128:192],
                            ps[:ss].rearrange("p (h c) -> p h c", h=6)[:, 1:6:2, :])

                # ---- C3: attention (this image) ----
                hv_i = hvp.tile([128, DT, S], BF16, tag="hv")
                for hpair in range(NH // 2):
                    e_pair = []
                    for h01 in range(2):
                        h = 2 * hpair + h01
                        p0 = h01 * 64
                        e_t = ep.tile([128, ST, QCH, QW], BF16, tag="E",
                                      name=f"E_{h01}")
                        e_pair.append(e_t)
                        for (kt, row0, ss) in _stiles(0):
                            sc = scps.tile([128, 2, 512], F32, name="scps",
                                           tag="s")
                            for qc in range(QCH):
                                nc.tensor.matmul(
                                    sc[0:ss, qc, 0:QW],
                                    qk_i[p0:p0 + 64, DT + hpair,
                                         kt * 128:kt * 128 + ss],
                                    qk_i[p0:p0 + 64, hpair,
                                         qc * QW:(qc + 1) * QW],
                                    start=True, stop=True)
                            nc.scalar.activation(
                                e_t[0:ss, kt, :, :], sc[0:ss, :, 0:QW],
                                AF.Exp, scale=0.125)
                    for h01 in range(2):
                        e_t = e_pair[h01]
                        p0 = h01 * 64
                        # even head: stationary [V|1] -> rows 0:64 hv, 64 d
                        # odd head: stationary [1|junk|V] -> row 0 d,
                        #           rows 64:128 hv
                        nr = 65 if h01 == 0 else 128
                        c0 = 0 if h01 == 0 else 64
                        dr = 64 if h01 == 0 else 0
                        for qc in range(QCH):
                            hv_ps = hps.tile([128, 512], F32, name="hvps",
                                             tag="h")
                            for (kt, row0, ss) in _stiles(0):
                                nc.tensor.matmul(
                                    hv_ps[0:nr, 0:QW],
                                    v65[0:ss, kt, hpair, c0:c0 + nr],
                                    e_t[0:ss, kt, qc, :],
                                    start=(kt == 0), stop=(kt == ST - 1))
                            r_sb = rp.tile([128, QW], F32, tag="rsb")
                            nc.vector.reciprocal(r_sb[dr:dr + 1, :],
                                                 hv_ps[dr:dr + 1, 0:QW])
                            # partition_broadcast HW ucode reads/writes from
                            # absolute partition 0 (AP partition offsets are
                            # ignored) - shift the even head's d row down first
                            if dr != 0:
                                r0 = rp.tile([1, QW], F32, tag="r0")
                                nc.sync.dma_start(r0[0:1, :],
                                                  r_sb[dr:dr + 1, :])
                                src = r0[0:1, :]
                            else:
                                src = r_sb[0:1, :]
                            r_b = bcp.tile([128, QW], F32, tag="rb")
                            nc.gpsimd.partition_broadcast(
                                r_b[0:p0 + 64, :], src, channels=p0 + 64)
                            nc.vector.tensor_mul(
                                hv_i[p0:p0 + 64, hpair, qc * QW:(qc + 1) * QW],
                                hv_ps[p0:p0 + 64, 0:QW],
                                r_b[p0:p0 + 64, :])

                # ---- C4: Wh + bias + residual -> nxt (this image) ----
                for mt in range(DT):
                    for ch in range(QCH):
                        ps = pps.tile([128, 384], F32, name="ops", tag="p")
                        for et in range(DT):
                            nc.tensor.matmul(
                                ps[:, 0:QW], wh_sb[:, et, mt * 128:(mt + 1) * 128],
                                hv_i[:, et, ch * QW:(ch + 1) * QW],
                                start=(et == 0), stop=(et == DT - 1))
                        t_f = tmpp.tile([128, 384], F32, tag="t46b")
                        nc.scalar.activation(t_f[:, 0:QW], ps[:, 0:QW],
                                             AF.Identity,
                                             bias=whb_sb[:, mt:mt + 1])
                        nc.vector.tensor_add(
                            res[:, mt, img * S + ch * QW: img * S + (ch + 1) * QW],
                            t_f[:, 0:QW],
                            res[:, mt, img * S + ch * QW: img * S + (ch + 1) * QW])

            # ---- C5: LayerNorm(res) -> xn (bf16); f32r (tf32) stats ----
            xn = xnp.tile([128, DT, T], BF16, tag="xn")
            with tc.tile_pool(name="stps", bufs=1, space="PSUM") as stps:
                for ch in range(TCH):
                    sq = xnp.tile([128, DT, 384], F32R, tag="sq")
                    for kt in range(DT):
                        nc.vector.tensor_mul(sq[:, kt, :],
                                             res[:, kt, ch * 384:(ch + 1) * 384],
                                             res[:, kt, ch * 384:(ch + 1) * 384])
                    st0 = stps.tile([1, 384], F32, tag="st0")
                    st1 = stps.tile([1, 384], F32, tag="st1")
                    for kt in range(DT):
                        nc.tensor.matmul(
                            st0[:], ones_f[:],
                            res[:, kt, ch * 384:(ch + 1) * 384],
                            start=(kt == 0), stop=(kt == DT - 1))
                        nc.tensor.matmul(
                            st1[:], ones_f[:], sq[:, kt, :],
                            start=(kt == 0), stop=(kt == DT - 1))
                    mom = smallp.tile([1, 384], F32, tag="mom")
                    nc.scalar.mul(mom[:], st0[:], 1.0 / D)
                    msq = smallp.tile([1, 384], F32, tag="msq")
                    nc.vector.tensor_mul(msq[:], mom[:], mom[:])
                    ex2 = smallp.tile([1, 384], F32, tag="ex2")
                    nc.scalar.mul(ex2[:], st1[:], 1.0 / D)
                    var = smallp.tile([1, 384], F32, tag="var")
                    nc.vector.tensor_sub(var[:], ex2[:], msq[:])
                    nc.scalar.activation(var[:], var[:], AF.Sqrt, bias=eps2[:])
                    rstd = smallp.tile([1, 384], F32, tag="rstd")
                    nc.vector.reciprocal(rstd[:], var[:])
                    m_b = bcp.tile([128, 384], F32, tag="mb")
                    nc.gpsimd.partition_broadcast(m_b[:], mom[0:1, :])
                    r_b = bcp.tile([128, 384], F32, tag="rb2")
                    nc.gpsimd.partition_broadcast(r_b[:], rstd[0:1, :])
                    for mt in range(DT):
                        t_c = tmpp.tile([128, 384], F32, tag="t5a")
                        nc.vector.tensor_sub(t_c[:],
                                             res[:, mt, ch * 384:(ch + 1) * 384],
                                             m_b[:])
                        t_d = tmpp.tile([128, 384], F32, tag="t5b")
                        nc.vector.tensor_mul(t_d[:], t_c[:], r_b[:])
                        nc.scalar.activation(
                            xn[:, mt, ch * 384:(ch + 1) * 384], t_d[:],
                            AF.Identity, bias=l2b_sb[:, mt:mt + 1],
                            scale=l2s_sb[:, mt:mt + 1])

            # ---- C6: FFN + residual (in place on nxt) ----
            with tc.tile_pool(name="f2ps", bufs=1, space="PSUM") as f2ps, \
                 tc.tile_pool(name="gps", bufs=2, space="PSUM") as gps:
                for tch in range(TCH):
                    f2 = [f2ps.tile([128, 384], F32, tag=f"f2_{mt}", name=f"f2_{mt}")
                          for mt in range(DT)]
                    for ft in range(FT):
                        w1_sb = ffw.tile([128, DT, 128], BF16, tag="w1")
                        nc.sync.dma_start(
                            w1_sb[:],
                            w1[li, :, ft * 128:(ft + 1) * 128].rearrange(
                                "(t p) f -> p t f", p=128))
                        w2_sb = ffw.tile([128, D], BF16, tag="w2")
                        nc.sync.dma_start(w2_sb[:], w2[li, ft * 128:(ft + 1) * 128, :])
                        g_ps = gps.tile([128, 384], F32)
                        for kt in range(DT):
                            nc.tensor.matmul(
                                g_ps[:], w1_sb[:, kt, :],
                                xn[:, kt, tch * 384:(tch + 1) * 384],
                                start=(kt == 0), stop=(kt == DT - 1))
                        g_bf = gp.tile([128, 384], BF16, tag="gbf")
                        nc.scalar.activation(g_bf[:], g_ps[:], AF.Gelu,
                                             bias=b1_sb[:, ft:ft + 1])
                        for mt in range(DT):
                            nc.tensor.matmul(
                                f2[mt][:], w2_sb[:, mt * 128:(mt + 1) * 128],
                                g_bf[:], start=(ft == 0), stop=(ft == FT - 1))
                    for mt in range(DT):
                        t_f = tmpp.tile([128, 384], F32, tag="t46b")
                        nc.scalar.activation(t_f[:], f2[mt][:], AF.Identity,
                                             bias=b2_sb[:, mt:mt + 1])
                        nc.vector.tensor_add(
                            res[:, mt, tch * 384:(tch + 1) * 384],
                            t_f[:], res[:, mt, tch * 384:(tch + 1) * 384])

        # ================= Final LayerNorm -> out =================
        lnf_s = biasp.tile([128, DT], F32, tag="lnfs")
        nc.sync.dma_start(lnf_s[:], lnfs.rearrange("(t p) -> p t", p=128))
        lnf_b = biasp.tile([128, DT], F32, tag="lnfb")
        nc.sync.dma_start(lnf_b[:], lnfb.rearrange("(t p) -> p t", p=128))
        with tc.tile_pool(name="fout", bufs=1) as foutp, \
             tc.tile_pool(name="fstps", bufs=1, space="PSUM") as stps:
            for ch in range(TCH):
                sqf = xnp.tile([128, DT, 384], F32R, tag="sq")
                for kt in range(DT):
                    nc.vector.tensor_mul(sqf[:, kt, :],
                                         res[:, kt, ch * 384:(ch + 1) * 384],
                                         res[:, kt, ch * 384:(ch + 1) * 384])
                st0 = stps.tile([1, 384], F32, tag="st0")
                st1 = stps.tile([1, 384], F32, tag="st1")
                for kt in range(DT):
                    nc.tensor.matmul(
                        st0[:], ones_f[:],
                        res[:, kt, ch * 384:(ch + 1) * 384],
                        start=(kt == 0), stop=(kt == DT - 1))
                    nc.tensor.matmul(
                        st1[:], ones_f[:], sqf[:, kt, :],
                        start=(kt == 0), stop=(kt == DT - 1))
                mom = smallp.tile([1, 384], F32, tag="mom")
                nc.scalar.mul(mom[:], st0[:], 1.0 / D)
                msq = smallp.tile([1, 384], F32, tag="msq")
                nc.vector.tensor_mul(msq[:], mom[:], mom[:])
                ex2 = smallp.tile([1, 384], F32, tag="ex2")
                nc.scalar.mul(ex2[:], st1[:], 1.0 / D)
                var = smallp.tile([1, 384], F32, tag="var")
                nc.vector.tensor_sub(var[:], ex2[:], msq[:])
                nc.scalar.activation(var[:], var[:], AF.Sqrt, bias=epsf[:])
                rstd = smallp.tile([1, 384], F32, tag="rstd")
                nc.vector.reciprocal(rstd[:], var[:])
                m_b = bcp.tile([128, 384], F32, tag="mb")
                nc.gpsimd.partition_broadcast(m_b[:], mom[0:1, :])
                r_b = bcp.tile([128, 384], F32, tag="rb2")
                nc.gpsimd.partition_broadcast(r_b[:], rstd[0:1, :])
                for mt in range(DT):
                    t_c = tmpp.tile([128, 384], F32, tag="t5a")
                    nc.vector.tensor_sub(t_c[:],
                                         res[:, mt, ch * 384:(ch + 1) * 384], m_b[:])
                    t_d = tmpp.tile([128, 384], F32, tag="t5b")
                    nc.vector.tensor_mul(t_d[:], t_c[:], r_b[:])
                    o_sb = foutp.tile([128, 384], F32)
                    nc.scalar.activation(o_sb[:], t_d[:], AF.Identity,
                                         bias=lnf_b[:, mt:mt + 1],
                                         scale=lnf_s[:, mt:mt + 1])
                    c0 = ch * 384
                    for off in range(0, 384, 192):
                        col = c0 + off
                        img, s0 = divmod(col, S)
                        nc.sync.dma_start(
                            out[img, mt * 128:(mt + 1) * 128, s0:s0 + 192],
                            o_sb[:, off:off + 192])
    nc.finalize()
    return nc


def _pos_encoding(max_len, d):
    pos = np.arange(max_len)[:, None].astype(np.float32)
    div = np.exp(np.arange(0, d, 2).astype(np.float32) * (-np.log(10000.0) / d))
    pe = np.zeros((max_len, d), dtype=np.float32)
    pe[:, 0::2] = np.sin(pos * div)
    pe[:, 1::2] = np.cos(pos * div)
    return pe


_NC_CACHE = {}


def get_nc(n_layers=L):
    if n_layers not in _NC_CACHE:
        _NC_CACHE[n_layers] = build_kernel(n_layers)
    return _NC_CACHE[n_layers]


def make_in_maps(x, conv_w, conv_b, ln1_s, ln1_b, wq, wk, wv, wh, wh_b,
                 ln2_s, ln2_b, w1, b1, w2, b2, lnf_s, lnf_b):
    bf = ml_dtypes.bfloat16
    x = np.asarray(x, np.float32)
    patches = x.reshape(B, C, IMG // P, P, IMG // P, P)      # (B,C,ty,py,tx,px)
    patches = patches.transpose(0, 1, 3, 5, 2, 4).reshape(B, D, S).astype(bf)
    wckh = np.ascontiguousarray(
        np.asarray(conv_w, np.float32).reshape(D, D).T).astype(bf)
    pefh = np.ascontiguousarray(_pos_encoding(5000, D)[:S].T)
    shared = {
        "wck": wckh, "cb": np.asarray(conv_b, np.float32), "pef": pefh,
        "wq": np.asarray(wq, np.float32).astype(bf),
        "wk": np.asarray(wk, np.float32).astype(bf),
        "wv": np.asarray(wv, np.float32).astype(bf),
        "wh": np.asarray(wh, np.float32).astype(bf),
        "whb": np.asarray(wh_b, np.float32),
        "ln2s": np.asarray(ln2_s, np.float32),
        "ln2b": np.asarray(ln2_b, np.float32),
        "w1": np.asarray(w1, np.float32).astype(bf),
        "b1": np.asarray(b1, np.float32),
        "w2": np.asarray(w2, np.float32).astype(bf),
        "b2": np.asarray(b2, np.float32),
        "lnfs": np.asarray(lnf_s, np.float32),
        "lnfb": np.asarray(lnf_b, np.float32),
    }
    in_maps = []
    for c in range(NCORES):
        m = dict(shared)
        m["xp"] = np.ascontiguousarray(patches[c * NI:(c + 1) * NI])
        in_maps.append(m)
    return in_maps


def assemble_output(results):
    out = np.empty((B, S, D), np.float32)
    for c in range(NCORES):
        o = results[c]["out"]
        for i in range(NI):
            out[c * NI + i] = o[i].T
    return out


def kernel(**inputs) -> np.ndarray:
    nc = get_nc()
    in_maps = make_in_maps(**inputs)
    res = run_bass_kernel_spmd(nc, in_maps, core_ids=list(range(NCORES)))
    return assemble_output(res.results)
